# revision 21
# baseline (speedup 1.0000x reference)
"""CRF input-energy kernel for Trainium2 (8 NeuronCores, data-parallel on batch).

Computes out[B,T,U] = X @ kernel + bias, with left/right boundary energies
added at t=0 and t=T-1.

Strategy: pure data parallel — each of the 8 cores gets 8 of the 64 batch
sequences. The problem is memory-bound, so all device I/O is bf16 (the 2e-2
rel-err budget leaves ~7x margin): host-side we fold the bias and the
boundary energies directly into X via a least-squares solve (kernel [128,32]
has full column rank, so dx @ kernel = bias has an exact solution; dx is
added to every row, and the analogous solutions for the boundary vectors to
the t=0 / t=T-1 rows). The device kernel is then a pure matmul: X shard
relayouted d-major [D, R] (R = 8*4096 rows) so the contraction dim D=128
lands on SBUF partitions with contiguous DMA. The bf16 weight [128,32] is
replicated into all four 32-column PE groups via tile_position, X streams
through as the 512-wide moving operand, and the VectorEngine just casts each
PSUM tile f32->bf16 into SBUF for the store. The blocked transposed output
[128, R*U/128] is un-permuted on host.
"""

import sys
import types

import numpy as np
import ml_dtypes

import concourse.bass as bass
import concourse.tile as tile
from concourse import bacc, mybir
from concourse.bass import ds, ts
from concourse.bass_utils import run_bass_kernel_spmd


def _ensure_axon_hooks_importable():
    """bass_utils imports antenv.axon_hooks when tracing is requested (e.g. a
    stray BASS_TRACE=1 in the environment); some images lack that submodule.
    Register a functional stand-in so the import never hard-fails."""
    try:
        from antenv import axon_hooks  # noqa: F401
        return
    except ImportError:
        pass
    mod = types.ModuleType("antenv.axon_hooks")
    _hook = [None]
    mod.set_axon_ntff_profile_hook = lambda h: _hook.__setitem__(0, h)
    mod.get_axon_ntff_profile_hook = lambda: _hook[0]
    sys.modules["antenv.axon_hooks"] = mod
    import antenv

    antenv.axon_hooks = mod
    try:
        from trn_agent_boot.trn_boot import _ntff_profile_via_ctypes

        mod.set_axon_ntff_profile_hook(
            _ntff_profile_via_ctypes("/opt/axon/libaxon_pjrt.so")
        )
    except Exception:
        pass  # hook stays None -> bass_utils skips tracing gracefully


_ensure_axon_hooks_importable()

BF16 = ml_dtypes.bfloat16

B, T, D, U = 64, 4096, 128, 32
N_CORES = 8
SEQ_PER_CORE = B // N_CORES      # 8
R = SEQ_PER_CORE * T             # 32768 rows per core
PB = 128                         # SBUF partition count
MOV = 512                        # moving-operand width (rows per matmul)
GRP = PB // U                    # 4 PE column groups / PSUM partition groups
ROWS_PER_PS = GRP * MOV          # 2048 rows per psum tile
NPS = R // ROWS_PER_PS           # 16 psum tiles per core
CH = 8192                        # X rows per compute chunk (2 MB bf16)
NCH = R // CH                    # 4 chunks per core
PS_PER_CH = CH // ROWS_PER_PS    # 4 psum tiles per chunk
OUT_COLS = R * U // PB           # 8192 output columns on device

_NC_CACHE = {}


def _build():
    nc = bacc.Bacc(
        "TRN2", target_bir_lowering=False, debug=False, num_devices=N_CORES
    )
    f32 = mybir.dt.float32
    bf16 = mybir.dt.bfloat16
    xt = nc.dram_tensor("xt", [PB, R], bf16, kind="ExternalInput").ap()
    w = nc.dram_tensor("w", [PB, U], bf16, kind="ExternalInput").ap()
    out = nc.dram_tensor("out", [PB, OUT_COLS], bf16, kind="ExternalOutput").ap()

    with tile.TileContext(nc) as tc:
        with (
            tc.tile_pool(name="consts", bufs=1) as consts,
            tc.tile_pool(name="xin", bufs=NCH) as xin,
            tc.tile_pool(name="outp", bufs=4) as outp,
            tc.tile_pool(name="ps", bufs=6, space=bass.MemorySpace.PSUM) as psp,
        ):
            w_sb = consts.tile([PB, U], bf16)
            nc.scalar.dma_start(w_sb[:], w[:])

            for n in range(NCH):
                xt_t = xin.tile([PB, CH], bf16)
                if n == 0:
                    # split the first load so its first bytes land sooner
                    # (shorter descriptor-gen before the engines can start)
                    for lo, hw in ((0, 1024), (1024, 3072), (4096, 4096)):
                        nc.sync.dma_start(
                            xt_t[:, ds(lo, hw)], xt[:, ds(n * CH + lo, hw)]
                        )
                elif n == NCH - 1:
                    # split the final load so the last psum tile's matmuls
                    # only wait on a short transfer
                    for lo, hw in (
                        (0, 4096), (4096, 2048), (6144, 1024),
                        (7168, 512), (7680, 256), (7936, 256),
                    ):
                        nc.sync.dma_start(
                            xt_t[:, ds(lo, hw)], xt[:, ds(n * CH + lo, hw)]
                        )
                else:
                    nc.sync.dma_start(xt_t[:], xt[:, ds(n * CH, CH)])
                o_t = outp.tile([PB, PS_PER_CH * MOV], bf16)
                full_tiles = PS_PER_CH - 1 if n == NCH - 1 else PS_PER_CH
                for s in range(full_tiles):
                    ps = psp.tile([PB, MOV], f32)
                    for g in range(GRP):
                        blk = s * GRP + g  # 512-row block within chunk
                        nc.tensor.matmul(
                            ps[g * U : (g + 1) * U, :],
                            w_sb[:],
                            xt_t[:, ds(blk * MOV, MOV)],
                            start=True,
                            stop=True,
                            tile_position=(0, g * U),
                        )
                    nc.vector.tensor_copy(o_t[:, ts(s, MOV)], ps[:])
                if n == NCH - 1:
                    # the final 2048 rows become two half-width psum tiles
                    # [128, 256] so the very last matmul/cast/store chain is
                    # half-size; the store tail only covers the last 256 rows
                    base = full_tiles * GRP * MOV  # chunk row offset
                    for m in range(2):
                        ps = psp.tile([PB, MOV // 2], f32)
                        for g in range(GRP):
                            lo = base + (m * GRP + g) * (MOV // 2)
                            nc.tensor.matmul(
                                ps[g * U : (g + 1) * U, :],
                                w_sb[:],
                                xt_t[:, ds(lo, MOV // 2)],
                                start=True,
                                stop=True,
                                tile_position=(0, g * U),
                            )
                        nc.vector.tensor_copy(
                            o_t[:, ds(full_tiles * MOV + m * (MOV // 2), MOV // 2)],
                            ps[:],
                        )
                    headc = full_tiles * MOV + MOV // 2
                    nc.scalar.dma_start(
                        out[:, ds(n * PS_PER_CH * MOV, headc)], o_t[:, ds(0, headc)]
                    )
                    nc.scalar.dma_start(
                        out[:, ds(n * PS_PER_CH * MOV + headc, MOV // 2)],
                        o_t[:, ds(headc, MOV // 2)],
                    )
                else:
                    nc.scalar.dma_start(
                        out[:, ds(n * PS_PER_CH * MOV, PS_PER_CH * MOV)], o_t[:]
                    )
    nc.compile()
    return nc


def _get_nc():
    if "nc" not in _NC_CACHE:
        _NC_CACHE["nc"] = _build()
    return _NC_CACHE["nc"]


def _make_in_maps(X, kern, bias, left_boundary, right_boundary):
    X = np.asarray(X, dtype=np.float32)
    w = np.asarray(kern, dtype=np.float64)
    bias = np.asarray(bias, dtype=np.float64)
    lb = np.asarray(left_boundary, dtype=np.float64)
    rb = np.asarray(right_boundary, dtype=np.float64)
    # kernel [D,U] has full column rank (U=32 < D=128), so dx @ kernel = v has
    # exact solutions; fold bias into every row of X and the boundary vectors
    # into the t=0 / t=T-1 rows, making the device kernel a pure matmul.
    dxs = np.linalg.lstsq(w.T, np.stack([bias, lb, rb], axis=1), rcond=None)[0]
    dxb, dxl, dxr = dxs[:, 0], dxs[:, 1], dxs[:, 2]
    X2 = X + dxb.astype(np.float32)
    X2[:, 0, :] += dxl.astype(np.float32)
    X2[:, -1, :] += dxr.astype(np.float32)
    wb = np.ascontiguousarray(np.asarray(kern, dtype=np.float32).astype(BF16))
    in_maps = []
    for c in range(N_CORES):
        Xc = X2[c * SEQ_PER_CORE : (c + 1) * SEQ_PER_CORE].reshape(R, D)
        xt = np.ascontiguousarray(Xc.T.astype(BF16))
        in_maps.append({"xt": xt, "w": wb})
    return in_maps


def _unshard(results):
    H = MOV // 2
    outs = []
    for c in range(N_CORES):
        o = np.asarray(results[c]["out"]).astype(np.float32)  # [128, OUT_COLS]
        # first NPS-1 full psum tiles: p = 32g + u ; col = 512k + c ;
        # row = 2048k + 512g + c
        nfull = NPS - 1
        head = (
            o[:, : nfull * MOV]
            .reshape(GRP, U, nfull, MOV)
            .transpose(2, 0, 3, 1)
            .reshape(nfull * GRP * MOV, U)
        )
        # last 2048 rows: two half-width tiles, row = 256g + c within each
        mini = [
            o[:, nfull * MOV + m * H : nfull * MOV + (m + 1) * H]
            .reshape(GRP, U, H)
            .transpose(0, 2, 1)
            .reshape(GRP * H, U)
            for m in range(2)
        ]
        e = np.concatenate([head] + mini, axis=0).reshape(SEQ_PER_CORE, T, U)
        outs.append(e)
    return np.concatenate(outs, axis=0)


def _run(inputs, trace=False, trace_cores=None):
    nc = _get_nc()
    in_maps = _make_in_maps(
        inputs["X"],
        inputs["kernel"],
        inputs["bias"],
        inputs["left_boundary"],
        inputs["right_boundary"],
    )
    last_err = None
    for attempt in range(3):
        try:
            res = run_bass_kernel_spmd(
                nc, in_maps, list(range(N_CORES)), trace=trace,
                trace_cores=trace_cores,
            )
            return _unshard(res.results), res
        except Exception as e:  # transient device wedges (NRT_*) self-heal
            last_err = e
    raise last_err


def kernel(X, kernel, bias, left_boundary, right_boundary):
    out, _ = _run(
        {
            "X": X,
            "kernel": kernel,
            "bias": bias,
            "left_boundary": left_boundary,
            "right_boundary": right_boundary,
        }
    )
    return out


# revision 22
# speedup vs baseline: 1.1323x; 1.1323x over previous
"""CRF input-energy kernel for Trainium2 (8 NeuronCores, data-parallel on batch).

Computes out[B,T,U] = X @ kernel + bias, with left/right boundary energies
added at t=0 and t=T-1.

Strategy: pure data parallel — each of the 8 cores gets 8 of the 64 batch
sequences. The problem is memory-bound, so all device I/O is bf16 (the 2e-2
rel-err budget leaves ~7x margin): host-side we fold the bias and the
boundary energies directly into X via a least-squares solve (kernel [128,32]
has full column rank, so dx @ kernel = bias has an exact solution; dx is
added to every row, and the analogous solutions for the boundary vectors to
the t=0 / t=T-1 rows). The device kernel is then a pure matmul: X shard
relayouted d-major [D, R] (R = 8*4096 rows) so the contraction dim D=128
lands on SBUF partitions with contiguous DMA. The bf16 weight [128,32] is
replicated into all four 32-column PE groups via tile_position, X streams
through as the 512-wide moving operand, and the VectorEngine just casts each
PSUM tile f32->bf16 into SBUF for the store. The blocked transposed output
[128, R*U/128] is un-permuted on host.
"""

import sys
import types

import numpy as np
import ml_dtypes

import concourse.bass as bass
import concourse.tile as tile
from concourse import bacc, mybir
from concourse.bass import ds, ts
from concourse.bass_utils import run_bass_kernel_spmd


def _ensure_axon_hooks_importable():
    """bass_utils imports antenv.axon_hooks when tracing is requested (e.g. a
    stray BASS_TRACE=1 in the environment); some images lack that submodule.
    Register a functional stand-in so the import never hard-fails."""
    try:
        from antenv import axon_hooks  # noqa: F401
        return
    except ImportError:
        pass
    mod = types.ModuleType("antenv.axon_hooks")
    _hook = [None]
    mod.set_axon_ntff_profile_hook = lambda h: _hook.__setitem__(0, h)
    mod.get_axon_ntff_profile_hook = lambda: _hook[0]
    sys.modules["antenv.axon_hooks"] = mod
    import antenv

    antenv.axon_hooks = mod
    try:
        from trn_agent_boot.trn_boot import _ntff_profile_via_ctypes

        mod.set_axon_ntff_profile_hook(
            _ntff_profile_via_ctypes("/opt/axon/libaxon_pjrt.so")
        )
    except Exception:
        pass  # hook stays None -> bass_utils skips tracing gracefully


_ensure_axon_hooks_importable()

BF16 = ml_dtypes.bfloat16

B, T, D, U = 64, 4096, 128, 32
N_CORES = 8
SEQ_PER_CORE = B // N_CORES      # 8
R = SEQ_PER_CORE * T             # 32768 rows per core
PB = 128                         # SBUF partition count
MOV = 512                        # moving-operand width (rows per matmul)
GRP = PB // U                    # 4 PE column groups / PSUM partition groups
ROWS_PER_PS = GRP * MOV          # 2048 rows per psum tile
NPS = R // ROWS_PER_PS           # 16 psum tiles per core
CH = 8192                        # X rows per compute chunk (2 MB bf16)
NCH = R // CH                    # 4 chunks per core
PS_PER_CH = CH // ROWS_PER_PS    # 4 psum tiles per chunk
OUT_COLS = R * U // PB           # 8192 output columns on device

_NC_CACHE = {}


def _build():
    nc = bacc.Bacc(
        "TRN2", target_bir_lowering=False, debug=False, num_devices=N_CORES
    )
    f32 = mybir.dt.float32
    bf16 = mybir.dt.bfloat16
    xt = nc.dram_tensor("xt", [PB, R], bf16, kind="ExternalInput").ap()
    w = nc.dram_tensor("w", [PB, U], bf16, kind="ExternalInput").ap()
    out = nc.dram_tensor("out", [PB, OUT_COLS], bf16, kind="ExternalOutput").ap()

    with tile.TileContext(nc) as tc:
        with (
            tc.tile_pool(name="consts", bufs=1) as consts,
            tc.tile_pool(name="xin", bufs=NCH) as xin,
            tc.tile_pool(name="outp", bufs=4) as outp,
            tc.tile_pool(name="ps", bufs=6, space=bass.MemorySpace.PSUM) as psp,
        ):
            w_sb = consts.tile([PB, U], bf16)
            nc.scalar.dma_start(w_sb[:], w[:])

            for n in range(NCH):
                xt_t = xin.tile([PB, CH], bf16)
                if n == 0:
                    # split the first load so its first bytes land sooner
                    # (shorter descriptor-gen before the engines can start)
                    for lo, hw in ((0, 1024), (1024, 3072), (4096, 4096)):
                        nc.sync.dma_start(
                            xt_t[:, ds(lo, hw)], xt[:, ds(n * CH + lo, hw)]
                        )
                elif n == NCH - 1:
                    # split the final load so the last psum tile's matmuls
                    # only wait on a short transfer
                    for lo, hw in (
                        (0, 4096), (4096, 2048), (6144, 1024),
                        (7168, 512), (7680, 256), (7936, 256),
                    ):
                        nc.sync.dma_start(
                            xt_t[:, ds(lo, hw)], xt[:, ds(n * CH + lo, hw)]
                        )
                else:
                    nc.sync.dma_start(xt_t[:], xt[:, ds(n * CH, CH)])
                o_t = outp.tile([PB, PS_PER_CH * MOV], bf16)
                full_tiles = PS_PER_CH - 1 if n == NCH - 1 else PS_PER_CH
                for s in range(full_tiles):
                    ps = psp.tile([PB, MOV], f32)
                    for g in range(GRP):
                        blk = s * GRP + g  # 512-row block within chunk
                        nc.tensor.matmul(
                            ps[g * U : (g + 1) * U, :],
                            w_sb[:],
                            xt_t[:, ds(blk * MOV, MOV)],
                            start=True,
                            stop=True,
                            tile_position=(0, g * U),
                        )
                    nc.vector.tensor_copy(o_t[:, ts(s, MOV)], ps[:])
                if n == NCH - 1:
                    # the final 2048 rows become two half-width psum tiles
                    # [128, 256] so the very last matmul/cast/store chain is
                    # half-size; the store tail only covers the last 256 rows
                    base = full_tiles * GRP * MOV  # chunk row offset
                    for m in range(2):
                        ps = psp.tile([PB, MOV // 2], f32)
                        for g in range(GRP):
                            lo = base + (m * GRP + g) * (MOV // 2)
                            nc.tensor.matmul(
                                ps[g * U : (g + 1) * U, :],
                                w_sb[:],
                                xt_t[:, ds(lo, MOV // 2)],
                                start=True,
                                stop=True,
                                tile_position=(0, g * U),
                            )
                        nc.vector.tensor_copy(
                            o_t[:, ds(full_tiles * MOV + m * (MOV // 2), MOV // 2)],
                            ps[:],
                        )
                    # three-way store split: full tiles early on scalar,
                    # mini tile 1 on scalar, mini tile 2 on the sync queue
                    # (idle after loads) so the two final descriptor-gens
                    # run on different sequencers in parallel
                    headc = full_tiles * MOV
                    ob = n * PS_PER_CH * MOV
                    nc.scalar.dma_start(out[:, ds(ob, headc)], o_t[:, ds(0, headc)])
                    nc.scalar.dma_start(
                        out[:, ds(ob + headc, MOV // 2)],
                        o_t[:, ds(headc, MOV // 2)],
                    )
                    nc.sync.dma_start(
                        out[:, ds(ob + headc + MOV // 2, MOV // 2)],
                        o_t[:, ds(headc + MOV // 2, MOV // 2)],
                    )
                else:
                    nc.scalar.dma_start(
                        out[:, ds(n * PS_PER_CH * MOV, PS_PER_CH * MOV)], o_t[:]
                    )
    nc.compile()
    return nc


def _get_nc():
    if "nc" not in _NC_CACHE:
        _NC_CACHE["nc"] = _build()
    return _NC_CACHE["nc"]


def _make_in_maps(X, kern, bias, left_boundary, right_boundary):
    X = np.asarray(X, dtype=np.float32)
    w = np.asarray(kern, dtype=np.float64)
    bias = np.asarray(bias, dtype=np.float64)
    lb = np.asarray(left_boundary, dtype=np.float64)
    rb = np.asarray(right_boundary, dtype=np.float64)
    # kernel [D,U] has full column rank (U=32 < D=128), so dx @ kernel = v has
    # exact solutions; fold bias into every row of X and the boundary vectors
    # into the t=0 / t=T-1 rows, making the device kernel a pure matmul.
    dxs = np.linalg.lstsq(w.T, np.stack([bias, lb, rb], axis=1), rcond=None)[0]
    dxb, dxl, dxr = dxs[:, 0], dxs[:, 1], dxs[:, 2]
    X2 = X + dxb.astype(np.float32)
    X2[:, 0, :] += dxl.astype(np.float32)
    X2[:, -1, :] += dxr.astype(np.float32)
    wb = np.ascontiguousarray(np.asarray(kern, dtype=np.float32).astype(BF16))
    in_maps = []
    for c in range(N_CORES):
        Xc = X2[c * SEQ_PER_CORE : (c + 1) * SEQ_PER_CORE].reshape(R, D)
        xt = np.ascontiguousarray(Xc.T.astype(BF16))
        in_maps.append({"xt": xt, "w": wb})
    return in_maps


def _unshard(results):
    H = MOV // 2
    outs = []
    for c in range(N_CORES):
        o = np.asarray(results[c]["out"]).astype(np.float32)  # [128, OUT_COLS]
        # first NPS-1 full psum tiles: p = 32g + u ; col = 512k + c ;
        # row = 2048k + 512g + c
        nfull = NPS - 1
        head = (
            o[:, : nfull * MOV]
            .reshape(GRP, U, nfull, MOV)
            .transpose(2, 0, 3, 1)
            .reshape(nfull * GRP * MOV, U)
        )
        # last 2048 rows: two half-width tiles, row = 256g + c within each
        mini = [
            o[:, nfull * MOV + m * H : nfull * MOV + (m + 1) * H]
            .reshape(GRP, U, H)
            .transpose(0, 2, 1)
            .reshape(GRP * H, U)
            for m in range(2)
        ]
        e = np.concatenate([head] + mini, axis=0).reshape(SEQ_PER_CORE, T, U)
        outs.append(e)
    return np.concatenate(outs, axis=0)


def _run(inputs, trace=False, trace_cores=None):
    nc = _get_nc()
    in_maps = _make_in_maps(
        inputs["X"],
        inputs["kernel"],
        inputs["bias"],
        inputs["left_boundary"],
        inputs["right_boundary"],
    )
    last_err = None
    for attempt in range(3):
        try:
            res = run_bass_kernel_spmd(
                nc, in_maps, list(range(N_CORES)), trace=trace,
                trace_cores=trace_cores,
            )
            return _unshard(res.results), res
        except Exception as e:  # transient device wedges (NRT_*) self-heal
            last_err = e
    raise last_err


def kernel(X, kernel, bias, left_boundary, right_boundary):
    out, _ = _run(
        {
            "X": X,
            "kernel": kernel,
            "bias": bias,
            "left_boundary": left_boundary,
            "right_boundary": right_boundary,
        }
    )
    return out
